# revision 21
# baseline (speedup 1.0000x reference)
"""RBF kernel matrix on 8 TRN2 NeuronCores.

Computes out[i, j] = exp(-gamma * (||x_i||^2 + ||y_j||^2 - 2 x_i.y_j))
with gamma = softplus(MLP(x[0])) + 1e-6, as a Bass/Tile SPMD kernel.

Sharding: rows of x across the 8 cores (1024 rows each); y and the tiny
gamma-net params are replicated.  Each core computes its (1024, 8192)
slab in bf16 (upcast to f32 on the host); the host concatenates.

Per-core pipeline (v4, factorized + group-major software pipeline):
  exp(-g d^2) = exp(2g x.y - g|x_i|^2) * exp(-g|y_j|^2)
  - TensorE: the bf16 cross GEMM (2 K-chunk matmuls per 512-col block)
    plus one-time prep (gamma chain, x_sq selector matmuls + PE
    transpose for the ACT bias table, y_sq rows, R replication).
  - ScalarE: Exp drains [128, 2048] psum straight to bf16 with the
    -g*x_sq per-partition bias fused; also exponentiates the replicated
    R = exp(-g y_sq) table (one [128, 2048] group per col-group).
  - VectorE: one bf16 tensor_tensor multiply by R per output tile; also
    the gamma relu (fused add+max), squares, and -g scalings.
  - DMA: y arrives as 8 x 512 KB loads; bf16 stage tiles stream out.
  Col-group prep is emitted one group ahead of the matching main-loop
  block so every engine queue stays dense.
"""

import numpy as np
import ml_dtypes

import concourse.bacc as bacc
import concourse.bass as bass  # noqa: F401
import concourse.mybir as mybir
import concourse.tile as tile
from concourse.bass_utils import run_bass_kernel_spmd

N_CORES = 8
N, M, D = 8192, 8192, 256
N_SH = N // N_CORES  # rows of x per core
HID = 16
P = 128
KC = D // P  # k-chunks (2)

F32 = mybir.dt.float32
BF16 = mybir.dt.bfloat16
FP8 = mybir.dt.float8e4
AF = mybir.ActivationFunctionType
ALU = mybir.AluOpType

_NC = None
LAST_RESULT = None


def _ensure_ntff_hook():
    """Register an ``antenv.axon_hooks`` shim if the image lacks it."""
    import contextlib
    import ctypes
    import os
    import sys
    import types

    try:
        import antenv.axon_hooks  # noqa: F401
        return
    except ImportError:
        pass

    hook = None
    so_path = "/opt/axon/libaxon_pjrt.so"
    if os.path.exists(so_path):
        try:
            lib = ctypes.CDLL(so_path)
            if hasattr(lib, "axon_start_nrt_profile"):
                lib.axon_start_nrt_profile.argtypes = [
                    ctypes.POINTER(ctypes.c_int64), ctypes.c_size_t]
                lib.axon_start_nrt_profile.restype = ctypes.c_int64
                lib.axon_stop_nrt_profile.argtypes = [ctypes.c_char_p]
                lib.axon_stop_nrt_profile.restype = ctypes.c_int64

                @contextlib.contextmanager
                def _hook(output_dir, device_ids):
                    import jax
                    jax.devices()
                    if device_ids:
                        ids = (ctypes.c_int64 * len(device_ids))(*device_ids)
                        rc = lib.axon_start_nrt_profile(ids, len(device_ids))
                    else:
                        rc = lib.axon_start_nrt_profile(None, 0)
                    if rc != 0:
                        raise RuntimeError(f"axon_start_nrt_profile rc={rc}")
                    try:
                        yield
                    finally:
                        n = lib.axon_stop_nrt_profile(str(output_dir).encode())
                        if n <= 0:
                            print(f"ntff profile capture wrote {n} files",
                                  file=sys.stderr)

                hook = _hook
        except OSError:
            hook = None

    mod = types.ModuleType("antenv.axon_hooks")
    mod._hook = hook
    mod.get_axon_ntff_profile_hook = lambda: mod._hook

    def _set(h):
        mod._hook = h

    mod.set_axon_ntff_profile_hook = _set
    sys.modules["antenv.axon_hooks"] = mod
    try:
        import antenv
        antenv.axon_hooks = mod
    except ImportError:
        pass


_ensure_ntff_hook()


def _build_nc():
    nc = bacc.Bacc("TRN2", target_bir_lowering=False, debug=False,
                   num_devices=N_CORES)

    xt_d = nc.dram_tensor("xt", [KC, P, N_SH], BF16, kind="ExternalInput")
    yt_d = nc.dram_tensor("yt", [KC, P, M], FP8, kind="ExternalInput")
    x0_d = nc.dram_tensor("x0", [KC, P, 1], F32, kind="ExternalInput")
    w1t_d = nc.dram_tensor("w1t", [KC, P, HID], F32, kind="ExternalInput")
    b1_d = nc.dram_tensor("b1", [HID, 1], F32, kind="ExternalInput")
    w2t_d = nc.dram_tensor("w2t", [HID, 1], F32, kind="ExternalInput")
    b2_d = nc.dram_tensor("b2", [1, 1], F32, kind="ExternalInput")
    eye8_d = nc.dram_tensor("eye8", [8, 8], F32, kind="ExternalInput")
    out_d = nc.dram_tensor("out", [N_SH, M], BF16, kind="ExternalOutput")

    GCOL = 2048           # psum group columns (4 banks); also col-group size
    NG = M // GCOL        # 4 col groups

    with tile.TileContext(nc) as tc:
        with (
            tc.tile_pool(name="const", bufs=1) as const,
            tc.tile_pool(name="work", bufs=2) as work,
            tc.tile_pool(name="estage", bufs=3) as epool,
            tc.tile_pool(name="stage", bufs=3) as stage_pool,
            tc.tile_pool(name="psmm", bufs=2, space="PSUM") as psmm,
        ):
            # ---------------- input DMAs (xt/params first, y per group) --
            x0_sb = const.tile([P, KC, 1], F32)
            w1t_sb = const.tile([P, KC, HID], F32)
            b1_sb = const.tile([HID, 1], F32)
            w2t_sb = const.tile([HID, 1], F32)
            b2_sb = const.tile([1, 1], F32)
            eye8_sb = const.tile([8, 8], F32)
            xT_sb = const.tile([P, KC, N_SH], BF16)
            yt_sb = const.tile([P, KC, M], FP8)
            # Small gamma/x params first (they gate the gamma chain and the
            # engine instruction fetch shares the DMA path), then y.
            for k in range(KC):
                nc.sync.dma_start(x0_sb[:, k], x0_d[k])
                nc.sync.dma_start(w1t_sb[:, k], w1t_d[k])
                nc.sync.dma_start(xT_sb[:, k], xt_d[k])
            nc.sync.dma_start(b1_sb[:], b1_d[:])
            nc.sync.dma_start(w2t_sb[:], w2t_d[:])
            nc.sync.dma_start(b2_sb[:], b2_d[:])
            nc.sync.dma_start(eye8_sb[:], eye8_d[:])
            for g in range(NG):
                gsl = slice(g * GCOL, (g + 1) * GCOL)
                for k in range(KC):
                    nc.sync.dma_start(yt_sb[:, k, gsl], yt_d[k, :, gsl])

            # ---------------- gamma chain (critical path first) ---------
            x0_c = const.tile([P, KC, 1], F32)
            w1t_c = const.tile([P, KC, HID], F32)
            w2t_c = const.tile([HID, 1], F32)
            nc.vector.tensor_copy(x0_c[:], x0_sb[:])
            nc.vector.tensor_copy(w1t_c[:], w1t_sb[:])
            nc.vector.tensor_copy(w2t_c[:], w2t_sb[:])

            ones128 = const.tile([P, 1], BF16)     # ysq reduce lhsT
            nc.vector.memset(ones128[:], 1.0)
            ones1 = const.tile([1, P], BF16)       # replicate K=1 lhsT
            nc.vector.memset(ones1[:], 1.0)
            ones_row = const.tile([1, P], F32)     # gamma broadcast lhsT
            nc.vector.memset(ones_row[:], 1.0)
            sel8 = const.tile([P, 8, 8], BF16)     # one-hot selector lhsTs
            nc.vector.memset(sel8[:], 0.0)
            for q in range(8):
                nc.vector.memset(sel8[:, q, q:q + 1], 1.0)

            pp1 = psmm.tile([P, GCOL], F32, tag="mm")
            ps_h = pp1[0:HID, 0:1]
            for k in range(KC):
                nc.tensor.matmul(ps_h, w1t_c[:, k], x0_c[:, k],
                                 start=(k == 0), stop=(k == KC - 1))
            # relu on DVE: h = max(psum + b1, 0)
            h_sb = const.tile([HID, 1], F32)
            nc.vector.tensor_scalar(h_sb[:], ps_h, b1_sb[:], 0.0,
                                    ALU.add, ALU.max)

            ps_z = pp1[0:1, 512:513]
            nc.tensor.matmul(ps_z, w2t_c[:], h_sb[:], start=True, stop=True)
            # softplus(z) = ln(1 + e^z) computed with exp only (stays in one
            # ACT table set): u = e^z; t = 1+u; s0 = bit-trick log of t
            # (Schraudolph, +-0.015); one Newton step s = s0 + t*e^{-s0} - 1.
            import math
            K_LN = math.log(2.0) / (1 << 23)
            C_LN = -(127.0 * (1 << 23) - 0.0430 * (1 << 23)) * K_LN
            u_sb = const.tile([1, 1], F32)
            nc.scalar.activation(u_sb[:], ps_z, AF.Exp, bias=b2_sb[:])
            t_sb = const.tile([1, 1], F32)
            nc.vector.tensor_scalar(t_sb[:], u_sb[:], 1.0, None, ALU.add)
            s0_sb = const.tile([1, 1], F32)
            nc.vector.tensor_scalar(s0_sb[:], t_sb[:].bitcast(mybir.dt.uint32),
                                    K_LN, C_LN, ALU.mult, ALU.add)
            e_sb = const.tile([1, 1], F32)
            nc.scalar.activation(e_sb[:], s0_sb[:], AF.Exp, scale=-1.0)
            sm1_sb = const.tile([1, 1], F32)
            nc.vector.tensor_scalar(sm1_sb[:], s0_sb[:], 1.0, -1.0,
                                    ALU.mult, ALU.add)
            w_sb = const.tile([1, 1], F32)
            nc.vector.tensor_tensor(w_sb[:], t_sb[:], e_sb[:], ALU.mult)
            s_sb = const.tile([1, 1], F32)
            nc.vector.tensor_tensor(s_sb[:], sm1_sb[:], w_sb[:], ALU.add)

            ps_g = pp1[0:P, 1024:1025]
            nc.tensor.matmul(ps_g, ones_row[:], s_sb[:], start=True, stop=True)

            negg_f = const.tile([P, 1], F32)     # -gamma on every partition
            nc.vector.tensor_scalar(negg_f[:], ps_g, -1.0, -1e-6,
                                    ALU.mult, ALU.add)
            pos2g_f = const.tile([P, 1], F32)    # +2*gamma
            nc.vector.tensor_scalar(pos2g_f[:], ps_g, 2.0, 2e-6,
                                    ALU.mult, ALU.add)

            # xs = (2 gamma) * x^T  (bf16 cross lhsT) -- main-loop gate
            xs_sb = const.tile([P, KC, N_SH], BF16)
            nc.vector.tensor_scalar(xs_sb[:], xT_sb[:], pos2g_f[:],
                                    None, ALU.mult)

            # ---------------- x_sq -> per-partition bias ----------------
            sqx = const.tile([P, KC, N_SH], BF16)
            nc.vector.tensor_tensor(sqx[:], xT_sb[:], xT_sb[:], ALU.mult)
            pp2 = psmm.tile([P, GCOL], F32, tag="mm")
            xsq8 = pp2[0:8, 0:P]
            for q in range(8):
                for k in range(KC):
                    nc.tensor.matmul(
                        xsq8, sel8[:, q, :], sqx[:, k, q * P:(q + 1) * P],
                        start=(q == 0 and k == 0),
                        stop=(q == 7 and k == KC - 1))
            sx8 = const.tile([8, P], F32)
            nc.vector.tensor_scalar(sx8[:], xsq8, negg_f[0:8], None, ALU.mult)
            psT = pp2[0:P, 512:520]
            nc.tensor.transpose(psT, sx8[:], eye8_sb[:])
            biasT = const.tile([P, 8], F32)
            nc.vector.tensor_copy(biasT[:], psT)

            # PE warm-up: keep the array busy through the head so the HAM
            # clock gate is at 8/8 when the cross stream starts.
            wz = const.tile([1, 512], BF16)
            nc.vector.memset(wz[:], 0.0)
            wu_ps = pp2[0:P, 1024:1536]
            for _ in range(20):
                nc.tensor.matmul(wu_ps, ones1[:], wz[:],
                                 start=True, stop=True)

            # ---------------- group-major pipeline ----------------------
            ngy_sb = const.tile([1, M], BF16)      # -gamma * y_sq row
            R_sb = const.tile([P, M], BF16)        # exp(-g y_sq) replicated

            def prep_group(g):
                gsl = slice(g * GCOL, (g + 1) * GCOL)
                sqy = work.tile([P, KC, GCOL], BF16, tag="sqy")
                # group 0 gates the head -> fast DVE; rest on idle GPSIMD
                eng = nc.vector if g == 0 else nc.gpsimd
                eng.tensor_tensor(sqy[:], yt_sb[:, :, gsl],
                                  yt_sb[:, :, gsl], ALU.mult)
                ppc = psmm.tile([P, GCOL], F32, tag="mm")
                for j in range(GCOL // 512):
                    jsl = slice(j * 512, (j + 1) * 512)
                    for k in range(KC):
                        nc.tensor.matmul(ppc[0:1, jsl], ones128[:],
                                         sqy[:, k, jsl], start=(k == 0),
                                         stop=(k == KC - 1))
                nc.vector.tensor_scalar(ngy_sb[0:1, gsl], ppc[0:1, 0:GCOL],
                                        negg_f[0:1], None, ALU.mult)
                ppr = psmm.tile([P, GCOL], F32, tag="mm")
                for j in range(GCOL // 512):
                    csl = slice(g * GCOL + j * 512, g * GCOL + (j + 1) * 512)
                    nc.tensor.matmul(ppr[:, j * 512:(j + 1) * 512],
                                     ones1[:], ngy_sb[0:1, csl],
                                     start=True, stop=True)
                nc.scalar.activation(R_sb[:, gsl], ppr[:], AF.Exp)

            def main_group(g, inject_after_m0=None):
                gsl = slice(g * GCOL, (g + 1) * GCOL)
                for m in range(N_SH // P):
                    msl = slice(m * P, (m + 1) * P)
                    ps = psmm.tile([P, GCOL], F32, tag="mm")
                    for j in range(GCOL // 512):
                        jsl = slice(j * 512, (j + 1) * 512)
                        csl = slice(g * GCOL + j * 512,
                                    g * GCOL + (j + 1) * 512)
                        for k in range(KC):
                            nc.tensor.matmul(ps[:, jsl], xs_sb[:, k, msl],
                                             yt_sb[:, k, csl],
                                             start=(k == 0),
                                             stop=(k == KC - 1))
                    # tiny filler matmul: covers the PE's per-tile drain
                    # wait so the HAM activity window never sees an idle
                    nc.tensor.matmul(wu_ps[:, 0:P], ones1[:], wz[0:1, 0:P],
                                     start=True, stop=True)
                    estage = epool.tile([P, GCOL], BF16, tag="e")
                    nc.scalar.activation(estage[:], ps[:], AF.Exp,
                                         bias=biasT[:, m:m + 1])
                    stage = stage_pool.tile([P, GCOL], BF16, tag="out")
                    nc.vector.tensor_tensor(stage[:], estage[:],
                                            R_sb[:, gsl], ALU.mult)
                    nc.sync.dma_start(out_d[msl, gsl], stage[:])
                    if m == 0 and inject_after_m0 is not None:
                        inject_after_m0()

            # Stagger: group g+2's prep is emitted just after the first
            # m-tile of group g, so its R table is ready long before the
            # ACT stream reaches group g+2.
            prep_group(0)
            prep_group(1)
            main_group(0, lambda: prep_group(2))
            main_group(1, lambda: prep_group(3))
            main_group(2)
            main_group(3)
    nc.compile()
    return nc


def _get_nc():
    global _NC
    if _NC is None:
        _NC = _build_nc()
    return _NC


def kernel(x, y, W1, b1, W2, b2):
    global LAST_RESULT
    x = np.asarray(x, dtype=np.float32)
    y = np.asarray(y, dtype=np.float32)
    bf = ml_dtypes.bfloat16

    yt = np.ascontiguousarray(y.T).reshape(KC, P, M).astype(ml_dtypes.float8_e4m3fn)
    x0 = np.ascontiguousarray(x[0]).reshape(KC, P, 1).astype(np.float32)
    w1t = np.ascontiguousarray(np.asarray(W1, np.float32).T).reshape(KC, P, HID)
    b1c = np.asarray(b1, np.float32).reshape(HID, 1)
    w2t = np.ascontiguousarray(np.asarray(W2, np.float32).T).reshape(HID, 1)
    b2c = np.asarray(b2, np.float32).reshape(1, 1)
    eye8 = np.eye(8, dtype=np.float32)

    in_maps = []
    for c in range(N_CORES):
        shard = x[c * N_SH:(c + 1) * N_SH]
        xt = np.ascontiguousarray(shard.T).reshape(KC, P, N_SH).astype(bf)
        in_maps.append({"xt": xt, "yt": yt, "x0": x0, "w1t": w1t,
                        "b1": b1c, "w2t": w2t, "b2": b2c, "eye8": eye8})

    nc = _get_nc()
    LAST_RESULT = run_bass_kernel_spmd(nc, in_maps, core_ids=list(range(N_CORES)))
    return np.concatenate([LAST_RESULT.results[c]["out"]
                           for c in range(N_CORES)], axis=0).astype(np.float32)


# revision 25
# speedup vs baseline: 1.1293x; 1.1293x over previous
"""RBF kernel matrix on 8 TRN2 NeuronCores.

Computes out[i, j] = exp(-gamma * (||x_i||^2 + ||y_j||^2 - 2 x_i.y_j))
with gamma = softplus(MLP(x[0])) + 1e-6, as a Bass/Tile SPMD kernel.

Sharding: rows of x across the 8 cores (1024 rows each); y and the tiny
gamma-net params are replicated.  Each core computes its (1024, 8192)
slab in bf16 (upcast to f32 on the host); the host concatenates.

Per-core pipeline (v4, factorized + group-major software pipeline):
  exp(-g d^2) = exp(2g x.y - g|x_i|^2) * exp(-g|y_j|^2)
  - TensorE: the bf16 cross GEMM (2 K-chunk matmuls per 512-col block)
    plus one-time prep (gamma chain, x_sq selector matmuls + PE
    transpose for the ACT bias table, y_sq rows, R replication).
  - ScalarE: Exp drains [128, 2048] psum straight to bf16 with the
    -g*x_sq per-partition bias fused; also exponentiates the replicated
    R = exp(-g y_sq) table (one [128, 2048] group per col-group).
  - VectorE: one bf16 tensor_tensor multiply by R per output tile; also
    the gamma relu (fused add+max), squares, and -g scalings.
  - DMA: y arrives as 8 x 512 KB loads; bf16 stage tiles stream out.
  Col-group prep is emitted one group ahead of the matching main-loop
  block so every engine queue stays dense.
"""

import numpy as np
import ml_dtypes

import concourse.bacc as bacc
import concourse.bass as bass  # noqa: F401
import concourse.mybir as mybir
import concourse.tile as tile
from concourse.bass_utils import run_bass_kernel_spmd

N_CORES = 8
N, M, D = 8192, 8192, 256
N_SH = N // N_CORES  # rows of x per core
HID = 16
P = 128
KC = D // P  # k-chunks (2)

F32 = mybir.dt.float32
BF16 = mybir.dt.bfloat16
FP8 = mybir.dt.float8e4
AF = mybir.ActivationFunctionType
ALU = mybir.AluOpType

_NC = None
LAST_RESULT = None


def _ensure_ntff_hook():
    """Register an ``antenv.axon_hooks`` shim if the image lacks it."""
    import contextlib
    import ctypes
    import os
    import sys
    import types

    try:
        import antenv.axon_hooks  # noqa: F401
        return
    except ImportError:
        pass

    hook = None
    so_path = "/opt/axon/libaxon_pjrt.so"
    if os.path.exists(so_path):
        try:
            lib = ctypes.CDLL(so_path)
            if hasattr(lib, "axon_start_nrt_profile"):
                lib.axon_start_nrt_profile.argtypes = [
                    ctypes.POINTER(ctypes.c_int64), ctypes.c_size_t]
                lib.axon_start_nrt_profile.restype = ctypes.c_int64
                lib.axon_stop_nrt_profile.argtypes = [ctypes.c_char_p]
                lib.axon_stop_nrt_profile.restype = ctypes.c_int64

                @contextlib.contextmanager
                def _hook(output_dir, device_ids):
                    import jax
                    jax.devices()
                    if device_ids:
                        ids = (ctypes.c_int64 * len(device_ids))(*device_ids)
                        rc = lib.axon_start_nrt_profile(ids, len(device_ids))
                    else:
                        rc = lib.axon_start_nrt_profile(None, 0)
                    if rc != 0:
                        raise RuntimeError(f"axon_start_nrt_profile rc={rc}")
                    try:
                        yield
                    finally:
                        n = lib.axon_stop_nrt_profile(str(output_dir).encode())
                        if n <= 0:
                            print(f"ntff profile capture wrote {n} files",
                                  file=sys.stderr)

                hook = _hook
        except OSError:
            hook = None

    mod = types.ModuleType("antenv.axon_hooks")
    mod._hook = hook
    mod.get_axon_ntff_profile_hook = lambda: mod._hook

    def _set(h):
        mod._hook = h

    mod.set_axon_ntff_profile_hook = _set
    sys.modules["antenv.axon_hooks"] = mod
    try:
        import antenv
        antenv.axon_hooks = mod
    except ImportError:
        pass


_ensure_ntff_hook()


def _build_nc():
    nc = bacc.Bacc("TRN2", target_bir_lowering=False, debug=False,
                   num_devices=N_CORES)

    xt_d = nc.dram_tensor("xt", [KC, P, N_SH], BF16, kind="ExternalInput")
    yt_d = nc.dram_tensor("yt", [KC, P, M], FP8, kind="ExternalInput")
    x0_d = nc.dram_tensor("x0", [KC, P, 1], F32, kind="ExternalInput")
    w1t_d = nc.dram_tensor("w1t", [KC, P, HID], F32, kind="ExternalInput")
    b1_d = nc.dram_tensor("b1", [HID, 1], F32, kind="ExternalInput")
    w2t_d = nc.dram_tensor("w2t", [HID, 1], F32, kind="ExternalInput")
    b2_d = nc.dram_tensor("b2", [1, 1], F32, kind="ExternalInput")
    eye8_d = nc.dram_tensor("eye8", [8, 8], F32, kind="ExternalInput")
    out_d = nc.dram_tensor("out", [N_SH, M], BF16, kind="ExternalOutput")

    GCOL = 2048           # psum group columns (4 banks); also col-group size
    NG = M // GCOL        # 4 col groups

    with tile.TileContext(nc) as tc:
        with (
            tc.tile_pool(name="const", bufs=1) as const,
            tc.tile_pool(name="work", bufs=2) as work,
            tc.tile_pool(name="estage", bufs=3) as epool,
            tc.tile_pool(name="stage", bufs=3) as stage_pool,
            tc.tile_pool(name="psmm", bufs=2, space="PSUM") as psmm,
        ):
            # ---------------- input DMAs (xt/params first, y per group) --
            x0_sb = const.tile([P, KC, 1], F32)
            w1t_sb = const.tile([P, KC, HID], F32)
            b1_sb = const.tile([HID, 1], F32)
            w2t_sb = const.tile([HID, 1], F32)
            b2_sb = const.tile([1, 1], F32)
            eye8_sb = const.tile([8, 8], F32)
            xT_sb = const.tile([P, KC, N_SH], BF16)
            yt_sb = const.tile([P, KC, M], FP8)
            # Small gamma/x params first (they gate the gamma chain and the
            # engine instruction fetch shares the DMA path), then y.
            for k in range(KC):
                nc.sync.dma_start(x0_sb[:, k], x0_d[k])
                nc.sync.dma_start(w1t_sb[:, k], w1t_d[k])
                nc.sync.dma_start(xT_sb[:, k], xt_d[k])
            nc.sync.dma_start(b1_sb[:], b1_d[:])
            nc.sync.dma_start(w2t_sb[:], w2t_d[:])
            nc.sync.dma_start(b2_sb[:], b2_d[:])
            nc.sync.dma_start(eye8_sb[:], eye8_d[:])
            for g in range(NG):
                gsl = slice(g * GCOL, (g + 1) * GCOL)
                for k in range(KC):
                    nc.sync.dma_start(yt_sb[:, k, gsl], yt_d[k, :, gsl])

            # ---------------- gamma chain (critical path first) ---------
            x0_c = const.tile([P, KC, 1], F32)
            w1t_c = const.tile([P, KC, HID], F32)
            w2t_c = const.tile([HID, 1], F32)
            nc.vector.tensor_copy(x0_c[:], x0_sb[:])
            nc.vector.tensor_copy(w1t_c[:], w1t_sb[:])
            nc.vector.tensor_copy(w2t_c[:], w2t_sb[:])

            ones128 = const.tile([P, 1], BF16)     # ysq reduce lhsT
            nc.vector.memset(ones128[:], 1.0)
            ones1 = const.tile([1, P], BF16)       # replicate K=1 lhsT
            nc.vector.memset(ones1[:], 1.0)
            ones_row = const.tile([1, P], F32)     # gamma broadcast lhsT
            nc.vector.memset(ones_row[:], 1.0)
            sel8 = const.tile([P, 8, 8], BF16)     # one-hot selector lhsTs
            nc.vector.memset(sel8[:], 0.0)
            for q in range(8):
                nc.vector.memset(sel8[:, q, q:q + 1], 1.0)

            pp1 = psmm.tile([P, GCOL], F32, tag="mm")
            ps_h = pp1[0:HID, 0:1]
            for k in range(KC):
                nc.tensor.matmul(ps_h, w1t_c[:, k], x0_c[:, k],
                                 start=(k == 0), stop=(k == KC - 1))
            # relu on DVE: h = max(psum + b1, 0)
            h_sb = const.tile([HID, 1], F32)
            nc.vector.tensor_scalar(h_sb[:], ps_h, b1_sb[:], 0.0,
                                    ALU.add, ALU.max)

            ps_z = pp1[0:1, 512:513]
            nc.tensor.matmul(ps_z, w2t_c[:], h_sb[:], start=True, stop=True)
            # softplus(z) = ln(1 + e^z) computed with exp only (stays in one
            # ACT table set): u = e^z; t = 1+u; s0 = bit-trick log of t
            # (Schraudolph, +-0.015); one Newton step s = s0 + t*e^{-s0} - 1.
            import math
            K_LN = math.log(2.0) / (1 << 23)
            C_LN = -(127.0 * (1 << 23) - 0.0430 * (1 << 23)) * K_LN
            u_sb = const.tile([1, 1], F32)
            nc.scalar.activation(u_sb[:], ps_z, AF.Exp, bias=b2_sb[:])
            t_sb = const.tile([1, 1], F32)
            nc.vector.tensor_scalar(t_sb[:], u_sb[:], 1.0, None, ALU.add)
            s0_sb = const.tile([1, 1], F32)
            nc.vector.tensor_scalar(s0_sb[:], t_sb[:].bitcast(mybir.dt.uint32),
                                    K_LN, C_LN, ALU.mult, ALU.add)
            e_sb = const.tile([1, 1], F32)
            nc.scalar.activation(e_sb[:], s0_sb[:], AF.Exp, scale=-1.0)
            sm1_sb = const.tile([1, 1], F32)
            nc.vector.tensor_scalar(sm1_sb[:], s0_sb[:], 1.0, -1.0,
                                    ALU.mult, ALU.add)
            w_sb = const.tile([1, 1], F32)
            nc.vector.tensor_tensor(w_sb[:], t_sb[:], e_sb[:], ALU.mult)
            s_sb = const.tile([1, 1], F32)
            nc.vector.tensor_tensor(s_sb[:], sm1_sb[:], w_sb[:], ALU.add)

            ps_g = pp1[0:P, 1024:1025]
            nc.tensor.matmul(ps_g, ones_row[:], s_sb[:], start=True, stop=True)

            negg_f = const.tile([P, 1], F32)     # -gamma on every partition
            nc.vector.tensor_scalar(negg_f[:], ps_g, -1.0, -1e-6,
                                    ALU.mult, ALU.add)
            pos2g_f = const.tile([P, 1], F32)    # +2*gamma
            nc.vector.tensor_scalar(pos2g_f[:], ps_g, 2.0, 2e-6,
                                    ALU.mult, ALU.add)

            # xs = (2 gamma) * x^T  (bf16 cross lhsT) -- main-loop gate
            xs_sb = const.tile([P, KC, N_SH], BF16)
            nc.vector.tensor_scalar(xs_sb[:], xT_sb[:], pos2g_f[:],
                                    None, ALU.mult)

            # ---------------- x_sq -> per-partition bias ----------------
            sqx = const.tile([P, KC, N_SH], BF16)
            nc.vector.tensor_tensor(sqx[:], xT_sb[:], xT_sb[:], ALU.mult)
            pp2 = psmm.tile([P, GCOL], F32, tag="mm")
            xsq8 = pp2[0:8, 0:P]
            for q in range(8):
                for k in range(KC):
                    nc.tensor.matmul(
                        xsq8, sel8[:, q, :], sqx[:, k, q * P:(q + 1) * P],
                        start=(q == 0 and k == 0),
                        stop=(q == 7 and k == KC - 1))
            sx8 = const.tile([8, P], F32)
            nc.vector.tensor_scalar(sx8[:], xsq8, negg_f[0:8], None, ALU.mult)
            psT = pp2[0:P, 512:520]
            nc.tensor.transpose(psT, sx8[:], eye8_sb[:])
            biasT = const.tile([P, 8], F32)
            nc.vector.tensor_copy(biasT[:], psT)



            # ---------------- group-major pipeline ----------------------
            ngy_sb = const.tile([1, M], BF16)      # -gamma * y_sq row
            R_sb = const.tile([P, M], BF16)        # exp(-g y_sq) replicated

            def prep_group(g):
                gsl = slice(g * GCOL, (g + 1) * GCOL)
                sqy = work.tile([P, KC, GCOL], BF16, tag="sqy")
                nc.vector.tensor_tensor(sqy[:], yt_sb[:, :, gsl],
                                        yt_sb[:, :, gsl], ALU.mult)
                ppc = psmm.tile([P, GCOL], F32, tag="mm")
                for j in range(GCOL // 512):
                    jsl = slice(j * 512, (j + 1) * 512)
                    for k in range(KC):
                        nc.tensor.matmul(ppc[0:1, jsl], ones128[:],
                                         sqy[:, k, jsl], start=(k == 0),
                                         stop=(k == KC - 1))
                nc.vector.tensor_scalar(ngy_sb[0:1, gsl], ppc[0:1, 0:GCOL],
                                        negg_f[0:1], None, ALU.mult)
                # R row = exp(-g y_sq), then replicate to all 128 partitions
                # with log-doubling SBUF->SBUF DMAs (no PSUM, no PE).
                nc.scalar.activation(R_sb[0:1, gsl], ngy_sb[0:1, gsl], AF.Exp)
                s = 1
                while s < P:
                    nc.sync.dma_start(R_sb[s:2 * s, gsl], R_sb[0:s, gsl])
                    s *= 2

            def main_group(g, inject_after_m0=None):
                gsl = slice(g * GCOL, (g + 1) * GCOL)
                for m in range(N_SH // P):
                    msl = slice(m * P, (m + 1) * P)
                    ps = psmm.tile([P, GCOL], F32, tag="mm")
                    for j in range(GCOL // 512):
                        jsl = slice(j * 512, (j + 1) * 512)
                        csl = slice(g * GCOL + j * 512,
                                    g * GCOL + (j + 1) * 512)
                        for k in range(KC):
                            nc.tensor.matmul(ps[:, jsl], xs_sb[:, k, msl],
                                             yt_sb[:, k, csl],
                                             start=(k == 0),
                                             stop=(k == KC - 1))
                    estage = epool.tile([P, GCOL], BF16, tag="e")
                    nc.scalar.activation(estage[:], ps[:], AF.Exp,
                                         bias=biasT[:, m:m + 1])
                    stage = stage_pool.tile([P, GCOL], BF16, tag="out")
                    nc.vector.tensor_tensor(stage[:], estage[:],
                                            R_sb[:, gsl], ALU.mult)
                    half = GCOL // 2
                    for hh in range(2):
                        nc.sync.dma_start(
                            out_d[msl, g * GCOL + hh * half:
                                  g * GCOL + (hh + 1) * half],
                            stage[:, hh * half:(hh + 1) * half])
                    if m == 0 and inject_after_m0 is not None:
                        inject_after_m0()

            # Stagger: group g+2's prep is emitted just after the first
            # m-tile of group g, so its R table is ready long before the
            # ACT stream reaches group g+2.
            prep_group(0)
            prep_group(1)
            main_group(0, lambda: prep_group(2))
            main_group(1, lambda: prep_group(3))
            main_group(2)
            main_group(3)
    nc.compile()
    return nc


def _get_nc():
    global _NC
    if _NC is None:
        _NC = _build_nc()
    return _NC


def kernel(x, y, W1, b1, W2, b2):
    global LAST_RESULT
    x = np.asarray(x, dtype=np.float32)
    y = np.asarray(y, dtype=np.float32)
    bf = ml_dtypes.bfloat16

    yt = np.ascontiguousarray(y.T).reshape(KC, P, M).astype(ml_dtypes.float8_e4m3fn)
    x0 = np.ascontiguousarray(x[0]).reshape(KC, P, 1).astype(np.float32)
    w1t = np.ascontiguousarray(np.asarray(W1, np.float32).T).reshape(KC, P, HID)
    b1c = np.asarray(b1, np.float32).reshape(HID, 1)
    w2t = np.ascontiguousarray(np.asarray(W2, np.float32).T).reshape(HID, 1)
    b2c = np.asarray(b2, np.float32).reshape(1, 1)
    eye8 = np.eye(8, dtype=np.float32)

    in_maps = []
    for c in range(N_CORES):
        shard = x[c * N_SH:(c + 1) * N_SH]
        xt = np.ascontiguousarray(shard.T).reshape(KC, P, N_SH).astype(bf)
        in_maps.append({"xt": xt, "yt": yt, "x0": x0, "w1t": w1t,
                        "b1": b1c, "w2t": w2t, "b2": b2c, "eye8": eye8})

    nc = _get_nc()
    LAST_RESULT = run_bass_kernel_spmd(nc, in_maps, core_ids=list(range(N_CORES)))
    return np.concatenate([LAST_RESULT.results[c]["out"]
                           for c in range(N_CORES)], axis=0).astype(np.float32)


# revision 26
# speedup vs baseline: 1.1560x; 1.0237x over previous
"""RBF kernel matrix on 8 TRN2 NeuronCores.

Computes out[i, j] = exp(-gamma * (||x_i||^2 + ||y_j||^2 - 2 x_i.y_j))
with gamma = softplus(MLP(x[0])) + 1e-6, as a Bass/Tile SPMD kernel.

Sharding: rows of x across the 8 cores (1024 rows each); y and the tiny
gamma-net params are replicated.  Each core computes its (1024, 8192)
slab in bf16 (upcast to f32 on the host); the host concatenates.

Per-core pipeline (v4, factorized + group-major software pipeline):
  exp(-g d^2) = exp(2g x.y - g|x_i|^2) * exp(-g|y_j|^2)
  - TensorE: the bf16 cross GEMM (2 K-chunk matmuls per 512-col block)
    plus one-time prep (gamma chain, x_sq selector matmuls + PE
    transpose for the ACT bias table, y_sq rows, R replication).
  - ScalarE: Exp drains [128, 2048] psum straight to bf16 with the
    -g*x_sq per-partition bias fused; also exponentiates the replicated
    R = exp(-g y_sq) table (one [128, 2048] group per col-group).
  - VectorE: one bf16 tensor_tensor multiply by R per output tile; also
    the gamma relu (fused add+max), squares, and -g scalings.
  - DMA: y arrives as 8 x 512 KB loads; bf16 stage tiles stream out.
  Col-group prep is emitted one group ahead of the matching main-loop
  block so every engine queue stays dense.
"""

import numpy as np
import ml_dtypes

import concourse.bacc as bacc
import concourse.bass as bass  # noqa: F401
import concourse.mybir as mybir
import concourse.tile as tile
from concourse.bass_utils import run_bass_kernel_spmd

N_CORES = 8
N, M, D = 8192, 8192, 256
N_SH = N // N_CORES  # rows of x per core
HID = 16
P = 128
KC = D // P  # k-chunks (2)

F32 = mybir.dt.float32
BF16 = mybir.dt.bfloat16
AF = mybir.ActivationFunctionType
ALU = mybir.AluOpType

_NC = None
LAST_RESULT = None


def _ensure_ntff_hook():
    """Register an ``antenv.axon_hooks`` shim if the image lacks it."""
    import contextlib
    import ctypes
    import os
    import sys
    import types

    try:
        import antenv.axon_hooks  # noqa: F401
        return
    except ImportError:
        pass

    hook = None
    so_path = "/opt/axon/libaxon_pjrt.so"
    if os.path.exists(so_path):
        try:
            lib = ctypes.CDLL(so_path)
            if hasattr(lib, "axon_start_nrt_profile"):
                lib.axon_start_nrt_profile.argtypes = [
                    ctypes.POINTER(ctypes.c_int64), ctypes.c_size_t]
                lib.axon_start_nrt_profile.restype = ctypes.c_int64
                lib.axon_stop_nrt_profile.argtypes = [ctypes.c_char_p]
                lib.axon_stop_nrt_profile.restype = ctypes.c_int64

                @contextlib.contextmanager
                def _hook(output_dir, device_ids):
                    import jax
                    jax.devices()
                    if device_ids:
                        ids = (ctypes.c_int64 * len(device_ids))(*device_ids)
                        rc = lib.axon_start_nrt_profile(ids, len(device_ids))
                    else:
                        rc = lib.axon_start_nrt_profile(None, 0)
                    if rc != 0:
                        raise RuntimeError(f"axon_start_nrt_profile rc={rc}")
                    try:
                        yield
                    finally:
                        n = lib.axon_stop_nrt_profile(str(output_dir).encode())
                        if n <= 0:
                            print(f"ntff profile capture wrote {n} files",
                                  file=sys.stderr)

                hook = _hook
        except OSError:
            hook = None

    mod = types.ModuleType("antenv.axon_hooks")
    mod._hook = hook
    mod.get_axon_ntff_profile_hook = lambda: mod._hook

    def _set(h):
        mod._hook = h

    mod.set_axon_ntff_profile_hook = _set
    sys.modules["antenv.axon_hooks"] = mod
    try:
        import antenv
        antenv.axon_hooks = mod
    except ImportError:
        pass


_ensure_ntff_hook()


def _build_nc():
    nc = bacc.Bacc("TRN2", target_bir_lowering=False, debug=False,
                   num_devices=N_CORES)

    xt_d = nc.dram_tensor("xt", [KC, P, N_SH], BF16, kind="ExternalInput")
    yt_d = nc.dram_tensor("yt", [KC, P, M], BF16, kind="ExternalInput")
    x0_d = nc.dram_tensor("x0", [KC, P, 1], F32, kind="ExternalInput")
    w1t_d = nc.dram_tensor("w1t", [KC, P, HID], F32, kind="ExternalInput")
    b1_d = nc.dram_tensor("b1", [HID, 1], F32, kind="ExternalInput")
    w2t_d = nc.dram_tensor("w2t", [HID, 1], F32, kind="ExternalInput")
    b2_d = nc.dram_tensor("b2", [1, 1], F32, kind="ExternalInput")
    eye8_d = nc.dram_tensor("eye8", [8, 8], F32, kind="ExternalInput")
    out_d = nc.dram_tensor("out", [N_SH, M], BF16, kind="ExternalOutput")

    GCOL = 2048           # psum group columns (4 banks); also col-group size
    NG = M // GCOL        # 4 col groups

    with tile.TileContext(nc) as tc:
        with (
            tc.tile_pool(name="const", bufs=1) as const,
            tc.tile_pool(name="work", bufs=2) as work,
            tc.tile_pool(name="estage", bufs=3) as epool,
            tc.tile_pool(name="stage", bufs=3) as stage_pool,
            tc.tile_pool(name="psmm", bufs=2, space="PSUM") as psmm,
        ):
            # ---------------- input DMAs (xt/params first, y per group) --
            x0_sb = const.tile([P, KC, 1], F32)
            w1t_sb = const.tile([P, KC, HID], F32)
            b1_sb = const.tile([HID, 1], F32)
            w2t_sb = const.tile([HID, 1], F32)
            b2_sb = const.tile([1, 1], F32)
            eye8_sb = const.tile([8, 8], F32)
            xT_sb = const.tile([P, KC, N_SH], BF16)
            yt_sb = const.tile([P, KC, M], BF16)
            for k in range(KC):
                nc.sync.dma_start(x0_sb[:, k], x0_d[k])
                nc.sync.dma_start(w1t_sb[:, k], w1t_d[k])
                nc.sync.dma_start(xT_sb[:, k], xt_d[k])
            nc.sync.dma_start(b1_sb[:], b1_d[:])
            nc.sync.dma_start(w2t_sb[:], w2t_d[:])
            nc.sync.dma_start(b2_sb[:], b2_d[:])
            nc.sync.dma_start(eye8_sb[:], eye8_d[:])
            for g in range(NG):
                gsl = slice(g * GCOL, (g + 1) * GCOL)
                for k in range(KC):
                    nc.sync.dma_start(yt_sb[:, k, gsl], yt_d[k, :, gsl])

            # ---------------- gamma chain (critical path first) ---------
            x0_c = const.tile([P, KC, 1], F32)
            w1t_c = const.tile([P, KC, HID], F32)
            w2t_c = const.tile([HID, 1], F32)
            nc.vector.tensor_copy(x0_c[:], x0_sb[:])
            nc.vector.tensor_copy(w1t_c[:], w1t_sb[:])
            nc.vector.tensor_copy(w2t_c[:], w2t_sb[:])

            ones128 = const.tile([P, 1], BF16)     # ysq reduce lhsT
            nc.vector.memset(ones128[:], 1.0)
            ones1 = const.tile([1, P], BF16)       # replicate K=1 lhsT
            nc.vector.memset(ones1[:], 1.0)
            ones_row = const.tile([1, P], F32)     # gamma broadcast lhsT
            nc.vector.memset(ones_row[:], 1.0)
            sel8 = const.tile([P, 8, 8], BF16)     # one-hot selector lhsTs
            nc.vector.memset(sel8[:], 0.0)
            for q in range(8):
                nc.vector.memset(sel8[:, q, q:q + 1], 1.0)

            pp1 = psmm.tile([P, GCOL], F32, tag="mm")
            ps_h = pp1[0:HID, 0:1]
            for k in range(KC):
                nc.tensor.matmul(ps_h, w1t_c[:, k], x0_c[:, k],
                                 start=(k == 0), stop=(k == KC - 1))
            # relu on DVE: h = max(psum + b1, 0)
            h_sb = const.tile([HID, 1], F32)
            nc.vector.tensor_scalar(h_sb[:], ps_h, b1_sb[:], 0.0,
                                    ALU.add, ALU.max)

            ps_z = pp1[0:1, 512:513]
            nc.tensor.matmul(ps_z, w2t_c[:], h_sb[:], start=True, stop=True)
            u_sb = const.tile([1, 1], F32)
            nc.scalar.activation(u_sb[:], ps_z, AF.Exp, bias=b2_sb[:])
            s_sb = const.tile([1, 1], F32)  # softplus(z) = ln(1 + e^z)
            nc.scalar.activation(s_sb[:], u_sb[:], AF.Ln, bias=1.0)

            ps_g = pp1[0:P, 1024:1025]
            nc.tensor.matmul(ps_g, ones_row[:], s_sb[:], start=True, stop=True)

            negg_f = const.tile([P, 1], F32)     # -gamma on every partition
            nc.vector.tensor_scalar(negg_f[:], ps_g, -1.0, -1e-6,
                                    ALU.mult, ALU.add)
            pos2g_f = const.tile([P, 1], F32)    # +2*gamma
            nc.vector.tensor_scalar(pos2g_f[:], ps_g, 2.0, 2e-6,
                                    ALU.mult, ALU.add)

            # xs = (2 gamma) * x^T  (bf16 cross lhsT) -- main-loop gate
            xs_sb = const.tile([P, KC, N_SH], BF16)
            nc.vector.tensor_scalar(xs_sb[:], xT_sb[:], pos2g_f[:],
                                    None, ALU.mult)

            # ---------------- x_sq -> per-partition bias ----------------
            sqx = const.tile([P, KC, N_SH], BF16)
            nc.vector.tensor_tensor(sqx[:], xT_sb[:], xT_sb[:], ALU.mult)
            pp2 = psmm.tile([P, GCOL], F32, tag="mm")
            xsq8 = pp2[0:8, 0:P]
            for q in range(8):
                for k in range(KC):
                    nc.tensor.matmul(
                        xsq8, sel8[:, q, :], sqx[:, k, q * P:(q + 1) * P],
                        start=(q == 0 and k == 0),
                        stop=(q == 7 and k == KC - 1))
            sx8 = const.tile([8, P], F32)
            nc.vector.tensor_scalar(sx8[:], xsq8, negg_f[0:8], None, ALU.mult)
            psT = pp2[0:P, 512:520]
            nc.tensor.transpose(psT, sx8[:], eye8_sb[:])
            biasT = const.tile([P, 8], F32)
            nc.vector.tensor_copy(biasT[:], psT)

            # ---------------- group-major pipeline ----------------------
            ngy_sb = const.tile([1, M], BF16)      # -gamma * y_sq row
            R_sb = const.tile([P, M], BF16)        # exp(-g y_sq) replicated

            def prep_group(g):
                gsl = slice(g * GCOL, (g + 1) * GCOL)
                sqy = work.tile([P, KC, GCOL], BF16, tag="sqy")
                nc.vector.tensor_tensor(sqy[:], yt_sb[:, :, gsl],
                                        yt_sb[:, :, gsl], ALU.mult)
                ppc = psmm.tile([P, GCOL], F32, tag="mm")
                for j in range(GCOL // 512):
                    jsl = slice(j * 512, (j + 1) * 512)
                    for k in range(KC):
                        nc.tensor.matmul(ppc[0:1, jsl], ones128[:],
                                         sqy[:, k, jsl], start=(k == 0),
                                         stop=(k == KC - 1))
                nc.vector.tensor_scalar(ngy_sb[0:1, gsl], ppc[0:1, 0:GCOL],
                                        negg_f[0:1], None, ALU.mult)
                ppr = psmm.tile([P, GCOL], F32, tag="mm")
                for j in range(GCOL // 512):
                    csl = slice(g * GCOL + j * 512, g * GCOL + (j + 1) * 512)
                    nc.tensor.matmul(ppr[:, j * 512:(j + 1) * 512],
                                     ones1[:], ngy_sb[0:1, csl],
                                     start=True, stop=True)
                nc.scalar.activation(R_sb[:, gsl], ppr[:], AF.Exp)

            def main_group(g):
                gsl = slice(g * GCOL, (g + 1) * GCOL)
                for m in range(N_SH // P):
                    msl = slice(m * P, (m + 1) * P)
                    ps = psmm.tile([P, GCOL], F32, tag="mm")
                    for j in range(GCOL // 512):
                        jsl = slice(j * 512, (j + 1) * 512)
                        csl = slice(g * GCOL + j * 512,
                                    g * GCOL + (j + 1) * 512)
                        for k in range(KC):
                            nc.tensor.matmul(ps[:, jsl], xs_sb[:, k, msl],
                                             yt_sb[:, k, csl],
                                             start=(k == 0),
                                             stop=(k == KC - 1))
                    estage = epool.tile([P, GCOL], BF16, tag="e")
                    nc.scalar.activation(estage[:], ps[:], AF.Exp,
                                         bias=biasT[:, m:m + 1])
                    stage = stage_pool.tile([P, GCOL], BF16, tag="out")
                    nc.vector.tensor_tensor(stage[:], estage[:],
                                            R_sb[:, gsl], ALU.mult)
                    nc.sync.dma_start(out_d[msl, gsl], stage[:])

            # Stagger: prep runs one group ahead of the main block.
            prep_group(0)
            prep_group(1)
            main_group(0)
            prep_group(2)
            main_group(1)
            prep_group(3)
            main_group(2)
            main_group(3)
    nc.compile()
    return nc


def _get_nc():
    global _NC
    if _NC is None:
        _NC = _build_nc()
    return _NC


def kernel(x, y, W1, b1, W2, b2):
    global LAST_RESULT
    x = np.asarray(x, dtype=np.float32)
    y = np.asarray(y, dtype=np.float32)
    bf = ml_dtypes.bfloat16

    yt = np.ascontiguousarray(y.T).reshape(KC, P, M).astype(bf)
    x0 = np.ascontiguousarray(x[0]).reshape(KC, P, 1).astype(np.float32)
    w1t = np.ascontiguousarray(np.asarray(W1, np.float32).T).reshape(KC, P, HID)
    b1c = np.asarray(b1, np.float32).reshape(HID, 1)
    w2t = np.ascontiguousarray(np.asarray(W2, np.float32).T).reshape(HID, 1)
    b2c = np.asarray(b2, np.float32).reshape(1, 1)
    eye8 = np.eye(8, dtype=np.float32)

    in_maps = []
    for c in range(N_CORES):
        shard = x[c * N_SH:(c + 1) * N_SH]
        xt = np.ascontiguousarray(shard.T).reshape(KC, P, N_SH).astype(bf)
        in_maps.append({"xt": xt, "yt": yt, "x0": x0, "w1t": w1t,
                        "b1": b1c, "w2t": w2t, "b2": b2c, "eye8": eye8})

    nc = _get_nc()
    LAST_RESULT = run_bass_kernel_spmd(nc, in_maps, core_ids=list(range(N_CORES)))
    return np.concatenate([LAST_RESULT.results[c]["out"]
                           for c in range(N_CORES)], axis=0).astype(np.float32)
